# revision 19
# baseline (speedup 1.0000x reference)
"""Multi-head attention block (B=2, N=2048, C=1024, H=16, D=64) on 8
Trainium2 NeuronCores.

Sharding: core c -> batch b = c//4, head-group g = c%4 (tensor-parallel over
heads within a batch, 4 heads per core). QKV weights are column-sharded,
w_proj row-sharded; each core emits a partial [N, C] projection which the host
sums per batch (row-parallel reduce done on host) and then adds b_proj.

Per-core kernel structure (build_attention_nc):
- QKV: xt slabs (512 tokens) x column-sharded weights -> q,k (transposed
  layout, head pairs split across partition halves) and v (token-major, with
  a ones-column appended per head for the softmax denominator).
- Attention: for each (slab, head-pair), St = K^T Q for both heads issue as
  K=64 matmuls in opposite PE row groups (tile_position (0,0) / (64,0)) so
  they stream concurrently through the array; one ACT exp per mc chunk
  covers both heads [128, 2, 512]; PV accumulates [v|1]^T P into po[65, 512]
  whose 65th row is the softmax denominator; DVE normalizes. The attention
  phase is ACT(exp)-bound (~151us standalone); po is double-buffered so the
  normalize tail never stalls the next unit's PV chain.
- Proj: ot x wp row-shard, partial outputs DMA'd out (host reduces).

All matmul operands are bf16 (PSUM accumulation stays fp32); tolerance is
2e-2 and measured error is ~2.5e-3. (fp8 x/w for QKV was tried and is
numerically dead: pre-matmul quantization noise accumulates through the
1024-deep contraction into ~7% error in v.)

The timed loop build (loop_iters=P) emits a software-pipelined body: the
persistent q/k/v buffers are doubled and each For_i iteration runs
[attn(buf0); proj; qkv->buf1; attn(buf1); proj; qkv->buf0] (2 logical
forwards per For_i iteration, prologue qkv->buf0 before the loop) with
staggered semaphore resets, so the QKV of the next forward overlaps the
proj/output-DMA tail of the current one.
"""

import sys

sys.path.insert(0, "/opt/trn_rl_repo")

import numpy as np


from contextlib import ExitStack

import concourse.bacc as bacc
import concourse.tile as tile
from concourse import mybir

F32 = mybir.dt.float32
BF16 = mybir.dt.bfloat16

N = 2048
C = 1024
HL = 4  # heads per core
D = 64
KC = C // 128  # 8 contraction chunks
NS = N // 512  # 4 n-supers / slabs
MC = N // 128  # 16 m-chunks


def build_attention_nc(
    mm_dtype=BF16,
    loop_iters=None,
    phases=(1, 1, 1),
    stag=True,
    pipe=True,  # software-pipelined double-buffered loop body (loop builds only)
):
    nc = bacc.Bacc(None, target_bir_lowering=False, debug=False)

    MMDT = mm_dtype
    nbuf = 2 if (pipe and loop_iters is not None) else 1

    xt = nc.dram_tensor("xt", [C, N], MMDT, kind="ExternalInput")
    wq = nc.dram_tensor("wq", [C, 2, 128], MMDT, kind="ExternalInput")
    wk = nc.dram_tensor("wk", [C, 2, 128], MMDT, kind="ExternalInput")
    wv = nc.dram_tensor("wv", [C, 256], MMDT, kind="ExternalInput")
    bq = nc.dram_tensor("bq", [2, 128], F32, kind="ExternalInput")
    bk = nc.dram_tensor("bk", [2, 128], F32, kind="ExternalInput")
    bv = nc.dram_tensor("bv", [256], F32, kind="ExternalInput")
    wp = nc.dram_tensor("wp", [256, C], MMDT, kind="ExternalInput")
    onesv = nc.dram_tensor("onesv", [128, 64], MMDT, kind="ExternalInput")
    out = nc.dram_tensor("out", [N, C], F32, kind="ExternalOutput")

    with (
        tile.TileContext(nc) as tc,
        ExitStack() as ctx,
        nc.allow_low_precision(reason="bf16 matmuls within 2e-2 tolerance"),
    ):
        const = ctx.enter_context(tc.tile_pool(name="const", bufs=1))
        persist = ctx.enter_context(tc.tile_pool(name="persist", bufs=1))

        # --- constants / weights ---
        wq_sb = const.tile([128, KC, 256], MMDT)
        wk_sb = const.tile([128, KC, 256], MMDT)
        wv_sb = const.tile([128, KC, 256], MMDT)
        nc.sync.dma_start(out=wq_sb, in_=wq.rearrange("(kc p) j m -> p kc (j m)", p=128))
        nc.sync.dma_start(out=wk_sb, in_=wk.rearrange("(kc p) j m -> p kc (j m)", p=128))
        nc.sync.dma_start(out=wv_sb, in_=wv.rearrange("(kc p) m -> p kc m", p=128))
        bq_sb = const.tile([128, 2], F32)
        bk_sb = const.tile([128, 2], F32)
        nc.sync.dma_start(out=bq_sb, in_=bq.rearrange("j p -> p j"))
        nc.sync.dma_start(out=bk_sb, in_=bk.rearrange("j p -> p j"))
        bv_rep = const.tile([128, 256], F32)
        nc.sync.dma_start(out=bv_rep, in_=bv[:].unsqueeze(0).partition_broadcast(128))
        wp_sb = const.tile([128, 2, C], MMDT)
        nc.sync.dma_start(out=wp_sb, in_=wp.rearrange("(kc p) n -> p kc n", p=128))

        # --- persistent intermediates (nbuf copies for the pipelined loop) ---
        qt_sb = [persist.tile([128, 2, N], MMDT, name=f"qt{b}") for b in range(nbuf)]
        kt_sb = [persist.tile([128, 2, N], MMDT, name=f"kt{b}") for b in range(nbuf)]
        v_sb = [
            persist.tile([128, MC, HL, 65], MMDT, name=f"v{b}") for b in range(nbuf)
        ]  # [m, mc, head, d|1]
        ot_sb = persist.tile([128, 2, N], MMDT)  # [d(2 heads), hd-chunk, n]
        for b in range(nbuf):
            nc.sync.dma_start(
                out=v_sb[b][:, :, :, 64:65],
                in_=onesv.rearrange("p (a b c) -> p a b c", a=MC, b=HL, c=1),
            )
        # Phase-isolation profiling support: when an upstream phase is
        # disabled, zero-init the tiles it would have produced so downstream
        # phases are runnable. No-op for the normal (1,1,1) build.
        if not phases[0]:
            for b in range(nbuf):
                for t in (qt_sb[b], kt_sb[b]):
                    nc.vector.memset(t.bitcast(mybir.dt.uint16), 0)
                nc.vector.memset(v_sb[b][:, :, :, 0:64].bitcast(mybir.dt.uint16), 0)
        if not phases[1] and phases[2]:
            nc.vector.memset(ot_sb.bitcast(mybir.dt.uint16), 0)

        def phase_qkv(b):
            with (
                tc.tile_pool(name="xt_pool", bufs=2) as xt_pool,
                tc.tile_pool(name="qkv_ps", bufs=4, space="PSUM") as qkv_ps,
            ):
                for i in range(NS):
                    xts = xt_pool.tile([128, KC, 512], MMDT, tag="xts")
                    nc.sync.dma_start(
                        out=xts,
                        in_=xt.rearrange("(kc p) n -> p kc n", p=128)[
                            :, :, i * 512 : (i + 1) * 512
                        ],
                    )
                    for wsb, bsb, dst in (
                        (wq_sb, bq_sb, qt_sb[b]),
                        (wk_sb, bk_sb, kt_sb[b]),
                    ):
                        for j in range(2):
                            ps = qkv_ps.tile([128, 512], F32, tag="ps", name=f"qk_ps{i}{j}")
                            for kc in range(KC):
                                nc.tensor.matmul(
                                    ps,
                                    lhsT=wsb[:, kc, j * 128 : (j + 1) * 128],
                                    rhs=xts[:, kc, :],
                                    start=kc == 0,
                                    stop=kc == KC - 1,
                                )
                            nc.vector.tensor_scalar_add(
                                out=dst[:, j, i * 512 : (i + 1) * 512],
                                in0=ps,
                                scalar1=bsb[:, j : j + 1],
                            )
                    for jj in range(4):
                        ps = qkv_ps.tile([128, 256], F32, tag="psv", name=f"v_ps{i}{jj}")
                        for kc in range(KC):
                            nc.tensor.matmul(
                                ps,
                                lhsT=xts[:, kc, jj * 128 : (jj + 1) * 128],
                                rhs=wv_sb[:, kc, :],
                                start=kc == 0,
                                stop=kc == KC - 1,
                            )
                        mc = i * 4 + jj
                        nc.vector.tensor_add(
                            out=v_sb[b][:, mc, :, 0:64],
                            in0=ps.rearrange("p (h d) -> p h d", h=HL),
                            in1=bv_rep.rearrange("p (h d) -> p h d", h=HL),
                        )

        def phase_attn(b):
            # Head-pair processing: heads (2j, 2j+1) live in partition halves
            # 0:64 / 64:128 of qt/kt, so their K=64 St matmuls occupy disjoint
            # PE row groups (tile_position (0,0) / (64,0)) and stream
            # concurrently. Ring slot = one mc chunk x both heads
            # [128, 2, 512]; exp covers both heads in one ACT instruction.
            with (
                tc.tile_pool(name="st_ps", bufs=1, space="PSUM") as st_ps,
                tc.tile_pool(name="o_ps", bufs=2, space="PSUM") as o_ps,
                tc.tile_pool(name="p_pool", bufs=4) as p_pool,
                tc.tile_pool(name="r_pool", bufs=4) as r_pool,
            ):
                rings = [
                    st_ps.tile([128, 2, 512], F32, tag=f"ring{u}", name=f"ring{u}")
                    for u in range(2)
                ]
                for s in range(NS):
                    for j in range(2):
                        poA = o_ps.tile([128, 512], F32, tag="poA", name=f"poA_s{s}j{j}")
                        poB = o_ps.tile([128, 512], F32, tag="poB", name=f"poB_s{s}j{j}")

                        def emit_pv(prev):
                            pt, mc = prev
                            for hh, po in ((0, poA), (1, poB)):
                                nc.tensor.matmul(
                                    po[0:65, :],
                                    lhsT=v_sb[b][:, mc, 2 * j + hh, :],
                                    rhs=pt[:, hh, :],
                                    start=mc == 0,
                                    stop=mc == MC - 1,
                                )

                        prev = None
                        for mc in range(MC):
                            ring = rings[mc % 2]
                            for hh in range(2):
                                base = hh * 64
                                nc.tensor.matmul(
                                    ring[:, hh, :],
                                    lhsT=kt_sb[b][
                                        base : base + 64, j, mc * 128 : (mc + 1) * 128
                                    ],
                                    rhs=qt_sb[b][
                                        base : base + 64, j, s * 512 : (s + 1) * 512
                                    ],
                                    start=True,
                                    stop=True,
                                )
                            pt = p_pool.tile([128, 2, 512], MMDT, tag="pt")
                            nc.scalar.activation(
                                out=pt,
                                in_=ring,
                                func=mybir.ActivationFunctionType.Exp,
                            )
                            if prev is not None:
                                emit_pv(prev)
                            prev = (pt, mc)
                        emit_pv(prev)

                        for hh, po in ((0, poA), (1, poB)):
                            recip = r_pool.tile([1, 512], F32, tag=f"recip{hh}")
                            nc.vector.reciprocal(out=recip, in_=po[64:65, :])
                            rden_sb = r_pool.tile(
                                [64, 512], F32, tag=f"rden{hh}", name=f"rd_s{s}j{j}h{hh}"
                            )
                            nc.gpsimd.partition_broadcast(rden_sb, recip)
                            nc.vector.tensor_mul(
                                out=ot_sb[
                                    hh * 64 : hh * 64 + 64,
                                    j,
                                    s * 512 : (s + 1) * 512,
                                ],
                                in0=po[0:64, :],
                                in1=rden_sb,
                            )

        def phase_proj():
            with (
                tc.tile_pool(name="pj_ps", bufs=4, space="PSUM") as pj_ps,
                tc.tile_pool(name="out_pool", bufs=3) as out_pool,
            ):
                for nt in range(MC):
                    for cc in range(2):
                        ps = pj_ps.tile([128, 512], F32, tag="pjps", name=f"pj{nt}{cc}")
                        for hdc in range(2):
                            nc.tensor.matmul(
                                ps,
                                lhsT=ot_sb[:, hdc, nt * 128 : (nt + 1) * 128],
                                rhs=wp_sb[:, hdc, cc * 512 : (cc + 1) * 512],
                                start=hdc == 0,
                                stop=hdc == 1,
                            )
                        so = out_pool.tile([128, 512], F32, tag="so")
                        nc.vector.tensor_copy(out=so, in_=ps)
                        nc.sync.dma_start(
                            out=out[nt * 128 : (nt + 1) * 128, cc * 512 : (cc + 1) * 512],
                            in_=so,
                        )

        def body(b):
            if phases[0]:
                phase_qkv(b)
            if phases[1]:
                phase_attn(b)
            if phases[2]:
                phase_proj()

        if loop_iters is None:
            body(0)
        elif not pipe:
            with tc.For_i(0, loop_iters, 1, staggered_reset=stag):
                body(0)
        else:
            # Software-pipelined: prologue fills buf0; each For_i iteration
            # consumes one buffer and refills the other, twice.
            if phases[0]:
                phase_qkv(0)
            with tc.For_i(0, loop_iters, 1, staggered_reset=stag):
                for half in (0, 1):
                    if phases[1]:
                        phase_attn(half)
                    if phases[2]:
                        phase_proj()
                    if phases[0]:
                        phase_qkv(1 - half)

    nc.compile()
    return nc


EMBED_DIM = 1024
NUM_HEADS = 16
HEAD_DIM = 64
HPC = 4

_CACHE = {}


def _make_in_maps(x, w_qkv, b_qkv, w_proj):
    import ml_dtypes

    MM_NP = ml_dtypes.bfloat16
    scale = HEAD_DIM ** -0.5
    xts = [np.ascontiguousarray(x[b].T).astype(MM_NP) for b in range(2)]
    ones = np.ones((128, 64), MM_NP)
    in_maps = []
    for core in range(8):
        b, g = core // 4, core % 4
        cols = slice(g * HPC * HEAD_DIM, (g + 1) * HPC * HEAD_DIM)
        wq = (w_qkv[:, 0:C][:, cols] * scale).astype(MM_NP)
        wk = w_qkv[:, C : 2 * C][:, cols].astype(MM_NP)
        wv = w_qkv[:, 2 * C : 3 * C][:, cols].astype(MM_NP)
        bq = (b_qkv[0:C][cols] * scale).astype(np.float32)
        bk = b_qkv[C : 2 * C][cols].astype(np.float32)
        bvv = b_qkv[2 * C : 3 * C][cols].astype(np.float32)
        wp = np.ascontiguousarray(w_proj[cols.start : cols.stop, :]).astype(MM_NP)
        in_maps.append(
            {
                "xt": xts[b],
                "wq": np.ascontiguousarray(wq.reshape(C, 2, 128)),
                "wk": np.ascontiguousarray(wk.reshape(C, 2, 128)),
                "wv": np.ascontiguousarray(wv),
                "bq": np.ascontiguousarray(bq.reshape(2, 128)),
                "bk": np.ascontiguousarray(bk.reshape(2, 128)),
                "bv": np.ascontiguousarray(bvv),
                "wp": wp,
                "onesv": ones,
            }
        )
    return in_maps


def kernel(x, w_qkv, b_qkv, w_proj, b_proj):
    from concourse.bass_utils import run_bass_kernel_spmd

    x = np.asarray(x)
    w_qkv = np.asarray(w_qkv)
    b_qkv = np.asarray(b_qkv)
    w_proj = np.asarray(w_proj)
    b_proj = np.asarray(b_proj)

    if "nc" not in _CACHE:
        _CACHE["nc"] = build_attention_nc()
    nc = _CACHE["nc"]

    in_maps = _make_in_maps(x, w_qkv, b_qkv, w_proj)
    res = run_bass_kernel_spmd(nc, in_maps, core_ids=list(range(8)))

    outs = []
    for b in range(2):
        acc = res.results[b * 4]["out"].astype(np.float32).copy()
        for g in range(1, 4):
            acc += res.results[b * 4 + g]["out"]
        outs.append(acc)
    return (np.stack(outs) + b_proj.astype(np.float32)).astype(np.float32)


# revision 20
# speedup vs baseline: 1.0767x; 1.0767x over previous
"""Multi-head attention block (B=2, N=2048, C=1024, H=16, D=64) on 8
Trainium2 NeuronCores.

Sharding: core c -> batch b = c//4, head-group g = c%4 (tensor-parallel over
heads within a batch, 4 heads per core). QKV weights are column-sharded,
w_proj row-sharded; each core emits a partial [N, C] projection which the host
sums per batch (row-parallel reduce done on host) and then adds b_proj.

Per-core kernel structure (build_attention_nc):
- QKV: xt slabs (512 tokens) x column-sharded weights -> q,k (transposed
  layout, head pairs split across partition halves) and v (token-major, with
  a ones-column appended per head for the softmax denominator).
- Attention: for each (slab, head-pair), St = K^T Q for both heads issue as
  K=64 matmuls in opposite PE row groups (tile_position (0,0) / (64,0)) so
  they stream concurrently through the array; one ACT exp per mc chunk
  covers both heads [128, 2, 512]; PV accumulates [v|1]^T P into po[65, 512]
  whose 65th row is the softmax denominator; DVE normalizes. The attention
  phase is ACT(exp)-bound (~151us standalone); po is double-buffered so the
  normalize tail never stalls the next unit's PV chain.
- Proj: ot x wp row-shard, partial outputs DMA'd out (host reduces).

All matmul operands are bf16 (PSUM accumulation stays fp32); tolerance is
2e-2 and measured error is ~2.5e-3. (fp8 x/w for QKV was tried and is
numerically dead: pre-matmul quantization noise accumulates through the
1024-deep contraction into ~7% error in v.)

The timed loop build (loop_iters=P) emits a software-pipelined body: the
persistent q/k/v buffers are doubled and each For_i iteration runs
[attn(buf0); proj; qkv->buf1; attn(buf1); proj; qkv->buf0] (2 logical
forwards per For_i iteration, prologue qkv->buf0 before the loop) with
staggered semaphore resets, so the QKV of the next forward overlaps the
proj/output-DMA tail of the current one.
"""

import sys

sys.path.insert(0, "/opt/trn_rl_repo")

import numpy as np


from contextlib import ExitStack

import concourse.bacc as bacc
import concourse.tile as tile
from concourse import mybir

F32 = mybir.dt.float32
BF16 = mybir.dt.bfloat16

N = 2048
C = 1024
HL = 4  # heads per core
D = 64
KC = C // 128  # 8 contraction chunks
NS = N // 512  # 4 n-supers / slabs
MC = N // 128  # 16 m-chunks


def build_attention_nc(
    mm_dtype=BF16,
    loop_iters=None,
    phases=(1, 1, 1),
    stag=True,
    pipe=True,  # software-pipelined double-buffered loop body (loop builds only)
):
    nc = bacc.Bacc(None, target_bir_lowering=False, debug=False)

    MMDT = mm_dtype
    nbuf = 2 if (pipe and loop_iters is not None) else 1

    xt = nc.dram_tensor("xt", [C, N], MMDT, kind="ExternalInput")
    wq = nc.dram_tensor("wq", [C, 2, 128], MMDT, kind="ExternalInput")
    wk = nc.dram_tensor("wk", [C, 2, 128], MMDT, kind="ExternalInput")
    wv = nc.dram_tensor("wv", [C, 256], MMDT, kind="ExternalInput")
    bq = nc.dram_tensor("bq", [2, 128], F32, kind="ExternalInput")
    bk = nc.dram_tensor("bk", [2, 128], F32, kind="ExternalInput")
    bv = nc.dram_tensor("bv", [256], F32, kind="ExternalInput")
    wp = nc.dram_tensor("wp", [256, C], MMDT, kind="ExternalInput")
    onesv = nc.dram_tensor("onesv", [128, 64], MMDT, kind="ExternalInput")
    out = nc.dram_tensor("out", [N, C], F32, kind="ExternalOutput")

    with (
        tile.TileContext(nc) as tc,
        ExitStack() as ctx,
        nc.allow_low_precision(reason="bf16 matmuls within 2e-2 tolerance"),
    ):
        const = ctx.enter_context(tc.tile_pool(name="const", bufs=1))
        persist = ctx.enter_context(tc.tile_pool(name="persist", bufs=1))

        # --- constants / weights ---
        wq_sb = const.tile([128, KC, 256], MMDT)
        wk_sb = const.tile([128, KC, 256], MMDT)
        wv_sb = const.tile([128, KC, 256], MMDT)
        nc.sync.dma_start(out=wq_sb, in_=wq.rearrange("(kc p) j m -> p kc (j m)", p=128))
        nc.sync.dma_start(out=wk_sb, in_=wk.rearrange("(kc p) j m -> p kc (j m)", p=128))
        nc.sync.dma_start(out=wv_sb, in_=wv.rearrange("(kc p) m -> p kc m", p=128))
        bq_sb = const.tile([128, 2], F32)
        bk_sb = const.tile([128, 2], F32)
        nc.sync.dma_start(out=bq_sb, in_=bq.rearrange("j p -> p j"))
        nc.sync.dma_start(out=bk_sb, in_=bk.rearrange("j p -> p j"))
        bv_rep = const.tile([128, 256], F32)
        nc.sync.dma_start(out=bv_rep, in_=bv[:].unsqueeze(0).partition_broadcast(128))
        wp_sb = const.tile([128, 2, C], MMDT)
        nc.sync.dma_start(out=wp_sb, in_=wp.rearrange("(kc p) n -> p kc n", p=128))

        # --- persistent intermediates (nbuf copies for the pipelined loop) ---
        qt_sb = [persist.tile([128, 2, N], MMDT, name=f"qt{b}") for b in range(nbuf)]
        kt_sb = [persist.tile([128, 2, N], MMDT, name=f"kt{b}") for b in range(nbuf)]
        v_sb = [
            persist.tile([128, MC, HL, 65], MMDT, name=f"v{b}") for b in range(nbuf)
        ]  # [m, mc, head, d|1]
        ot_sb = persist.tile([128, 2, N], MMDT)  # [d(2 heads), hd-chunk, n]
        for b in range(nbuf):
            nc.sync.dma_start(
                out=v_sb[b][:, :, :, 64:65],
                in_=onesv.rearrange("p (a b c) -> p a b c", a=MC, b=HL, c=1),
            )
        # Phase-isolation profiling support: when an upstream phase is
        # disabled, zero-init the tiles it would have produced so downstream
        # phases are runnable. No-op for the normal (1,1,1) build.
        if not phases[0]:
            for b in range(nbuf):
                for t in (qt_sb[b], kt_sb[b]):
                    nc.vector.memset(t.bitcast(mybir.dt.uint16), 0)
                nc.vector.memset(v_sb[b][:, :, :, 0:64].bitcast(mybir.dt.uint16), 0)
        if not phases[1] and phases[2]:
            nc.vector.memset(ot_sb.bitcast(mybir.dt.uint16), 0)

        def phase_qkv(b):
            with (
                tc.tile_pool(name="xt_pool", bufs=2) as xt_pool,
                tc.tile_pool(name="qkv_ps", bufs=4, space="PSUM") as qkv_ps,
            ):
                for i in range(NS):
                    xts = xt_pool.tile([128, KC, 512], MMDT, tag="xts")
                    nc.sync.dma_start(
                        out=xts,
                        in_=xt.rearrange("(kc p) n -> p kc n", p=128)[
                            :, :, i * 512 : (i + 1) * 512
                        ],
                    )
                    for wsb, bsb, dst in (
                        (wq_sb, bq_sb, qt_sb[b]),
                        (wk_sb, bk_sb, kt_sb[b]),
                    ):
                        for j in range(2):
                            ps = qkv_ps.tile([128, 512], F32, tag="ps", name=f"qk_ps{i}{j}")
                            for kc in range(KC):
                                nc.tensor.matmul(
                                    ps,
                                    lhsT=wsb[:, kc, j * 128 : (j + 1) * 128],
                                    rhs=xts[:, kc, :],
                                    start=kc == 0,
                                    stop=kc == KC - 1,
                                )
                            nc.vector.tensor_scalar_add(
                                out=dst[:, j, i * 512 : (i + 1) * 512],
                                in0=ps,
                                scalar1=bsb[:, j : j + 1],
                            )
                    for jj in range(4):
                        ps = qkv_ps.tile([128, 256], F32, tag="psv", name=f"v_ps{i}{jj}")
                        for kc in range(KC):
                            nc.tensor.matmul(
                                ps,
                                lhsT=xts[:, kc, jj * 128 : (jj + 1) * 128],
                                rhs=wv_sb[:, kc, :],
                                start=kc == 0,
                                stop=kc == KC - 1,
                            )
                        mc = i * 4 + jj
                        nc.vector.tensor_add(
                            out=v_sb[b][:, mc, :, 0:64],
                            in0=ps.rearrange("p (h d) -> p h d", h=HL),
                            in1=bv_rep.rearrange("p (h d) -> p h d", h=HL),
                        )

        def phase_attn(b):
            # Head-pair processing: heads (2j, 2j+1) live in partition halves
            # 0:64 / 64:128 of qt/kt, so their K=64 St matmuls occupy disjoint
            # PE row groups (tile_position (0,0) / (64,0)) and stream
            # concurrently. Ring slot = one mc chunk x both heads
            # [128, 2, 512]; exp covers both heads in one ACT instruction.
            with (
                tc.tile_pool(name="st_ps", bufs=1, space="PSUM") as st_ps,
                tc.tile_pool(name="o_ps", bufs=2, space="PSUM") as o_ps,
                tc.tile_pool(name="p_pool", bufs=4) as p_pool,
                tc.tile_pool(name="r_pool", bufs=4) as r_pool,
            ):
                rings = [
                    st_ps.tile([128, 2, 512], F32, tag=f"ring{u}", name=f"ring{u}")
                    for u in range(2)
                ]

                # Flat global slot pipeline across all (s, j, mc): PV lags one
                # slot globally, so a new unit's St+exp issue before the
                # previous unit's last PV pair — no ACT bubble at unit
                # boundaries. The unit tail (normalize) is emitted right
                # after that unit's last PV retires.
                slots = [
                    (s, j, mc) for s in range(NS) for j in range(2) for mc in range(MC)
                ]
                po_of = {}

                def emit_pv(item):
                    s, j, mc, pt = item
                    poA, poB = po_of[(s, j)]
                    for hh, po in ((0, poA), (1, poB)):
                        nc.tensor.matmul(
                            po[0:65, :],
                            lhsT=v_sb[b][:, mc, 2 * j + hh, :],
                            rhs=pt[:, hh, :],
                            start=mc == 0,
                            stop=mc == MC - 1,
                        )

                def emit_tail(s, j):
                    poA, poB = po_of.pop((s, j))
                    for hh, po in ((0, poA), (1, poB)):
                        recip = r_pool.tile([1, 512], F32, tag=f"recip{hh}")
                        nc.vector.reciprocal(out=recip, in_=po[64:65, :])
                        rden_sb = r_pool.tile(
                            [64, 512], F32, tag=f"rden{hh}", name=f"rd_s{s}j{j}h{hh}"
                        )
                        nc.gpsimd.partition_broadcast(rden_sb, recip)
                        nc.vector.tensor_mul(
                            out=ot_sb[
                                hh * 64 : hh * 64 + 64,
                                j,
                                s * 512 : (s + 1) * 512,
                            ],
                            in0=po[0:64, :],
                            in1=rden_sb,
                        )

                prev = None
                for gi, (s, j, mc) in enumerate(slots):
                    if mc == 0:
                        po_of[(s, j)] = (
                            o_ps.tile([128, 512], F32, tag="poA", name=f"poA_s{s}j{j}"),
                            o_ps.tile([128, 512], F32, tag="poB", name=f"poB_s{s}j{j}"),
                        )
                    ring = rings[gi % 2]
                    for hh in range(2):
                        base = hh * 64
                        nc.tensor.matmul(
                            ring[:, hh, :],
                            lhsT=kt_sb[b][
                                base : base + 64, j, mc * 128 : (mc + 1) * 128
                            ],
                            rhs=qt_sb[b][
                                base : base + 64, j, s * 512 : (s + 1) * 512
                            ],
                            start=True,
                            stop=True,
                        )
                    pt = p_pool.tile([128, 2, 512], MMDT, tag="pt")
                    nc.scalar.activation(
                        out=pt,
                        in_=ring,
                        func=mybir.ActivationFunctionType.Exp,
                    )
                    if prev is not None:
                        emit_pv(prev)
                        ps_, pj_, pmc_ = prev[0], prev[1], prev[2]
                        if pmc_ == MC - 1:
                            emit_tail(ps_, pj_)
                    prev = (s, j, mc, pt)
                emit_pv(prev)
                emit_tail(prev[0], prev[1])

        def phase_proj():
            with (
                tc.tile_pool(name="pj_ps", bufs=4, space="PSUM") as pj_ps,
                tc.tile_pool(name="out_pool", bufs=3) as out_pool,
            ):
                for nt in range(MC):
                    for cc in range(2):
                        ps = pj_ps.tile([128, 512], F32, tag="pjps", name=f"pj{nt}{cc}")
                        for hdc in range(2):
                            nc.tensor.matmul(
                                ps,
                                lhsT=ot_sb[:, hdc, nt * 128 : (nt + 1) * 128],
                                rhs=wp_sb[:, hdc, cc * 512 : (cc + 1) * 512],
                                start=hdc == 0,
                                stop=hdc == 1,
                            )
                        so = out_pool.tile([128, 512], F32, tag="so")
                        nc.vector.tensor_copy(out=so, in_=ps)
                        nc.sync.dma_start(
                            out=out[nt * 128 : (nt + 1) * 128, cc * 512 : (cc + 1) * 512],
                            in_=so,
                        )

        def body(b):
            if phases[0]:
                phase_qkv(b)
            if phases[1]:
                phase_attn(b)
            if phases[2]:
                phase_proj()

        if loop_iters is None:
            body(0)
        elif not pipe:
            with tc.For_i(0, loop_iters, 1, staggered_reset=stag):
                body(0)
        else:
            # Software-pipelined: prologue fills buf0; each For_i iteration
            # consumes one buffer and refills the other, twice.
            if phases[0]:
                phase_qkv(0)
            with tc.For_i(0, loop_iters, 1, staggered_reset=stag):
                for half in (0, 1):
                    if phases[1]:
                        phase_attn(half)
                    if phases[2]:
                        phase_proj()
                    if phases[0]:
                        phase_qkv(1 - half)

    nc.compile()
    return nc


EMBED_DIM = 1024
NUM_HEADS = 16
HEAD_DIM = 64
HPC = 4

_CACHE = {}


def _make_in_maps(x, w_qkv, b_qkv, w_proj):
    import ml_dtypes

    MM_NP = ml_dtypes.bfloat16
    scale = HEAD_DIM ** -0.5
    xts = [np.ascontiguousarray(x[b].T).astype(MM_NP) for b in range(2)]
    ones = np.ones((128, 64), MM_NP)
    in_maps = []
    for core in range(8):
        b, g = core // 4, core % 4
        cols = slice(g * HPC * HEAD_DIM, (g + 1) * HPC * HEAD_DIM)
        wq = (w_qkv[:, 0:C][:, cols] * scale).astype(MM_NP)
        wk = w_qkv[:, C : 2 * C][:, cols].astype(MM_NP)
        wv = w_qkv[:, 2 * C : 3 * C][:, cols].astype(MM_NP)
        bq = (b_qkv[0:C][cols] * scale).astype(np.float32)
        bk = b_qkv[C : 2 * C][cols].astype(np.float32)
        bvv = b_qkv[2 * C : 3 * C][cols].astype(np.float32)
        wp = np.ascontiguousarray(w_proj[cols.start : cols.stop, :]).astype(MM_NP)
        in_maps.append(
            {
                "xt": xts[b],
                "wq": np.ascontiguousarray(wq.reshape(C, 2, 128)),
                "wk": np.ascontiguousarray(wk.reshape(C, 2, 128)),
                "wv": np.ascontiguousarray(wv),
                "bq": np.ascontiguousarray(bq.reshape(2, 128)),
                "bk": np.ascontiguousarray(bk.reshape(2, 128)),
                "bv": np.ascontiguousarray(bvv),
                "wp": wp,
                "onesv": ones,
            }
        )
    return in_maps


def kernel(x, w_qkv, b_qkv, w_proj, b_proj):
    from concourse.bass_utils import run_bass_kernel_spmd

    x = np.asarray(x)
    w_qkv = np.asarray(w_qkv)
    b_qkv = np.asarray(b_qkv)
    w_proj = np.asarray(w_proj)
    b_proj = np.asarray(b_proj)

    if "nc" not in _CACHE:
        _CACHE["nc"] = build_attention_nc()
    nc = _CACHE["nc"]

    in_maps = _make_in_maps(x, w_qkv, b_qkv, w_proj)
    res = run_bass_kernel_spmd(nc, in_maps, core_ids=list(range(8)))

    outs = []
    for b in range(2):
        acc = res.results[b * 4]["out"].astype(np.float32).copy()
        for g in range(1, 4):
            acc += res.results[b * 4 + g]["out"]
        outs.append(acc)
    return (np.stack(outs) + b_proj.astype(np.float32)).astype(np.float32)


# revision 26
# speedup vs baseline: 1.1192x; 1.0394x over previous
"""Multi-head attention block (B=2, N=2048, C=1024, H=16, D=64) on 8
Trainium2 NeuronCores.

Sharding: core c -> batch b = c//4, head-group g = c%4 (tensor-parallel over
heads within a batch, 4 heads per core). QKV weights are column-sharded,
w_proj row-sharded; each core emits a partial [N, C] projection which the host
sums per batch (row-parallel reduce done on host) and then adds b_proj.

Per-core kernel structure (build_attention_nc):
- QKV: xt slabs (512 tokens) x column-sharded weights -> q,k (transposed
  layout, head pairs split across partition halves) and v (token-major, with
  a ones-column appended per head for the softmax denominator).
- Attention: for each (slab, head-pair), St = K^T Q for both heads issue as
  K=64 matmuls in opposite PE row groups (tile_position (0,0) / (64,0)) so
  they stream concurrently through the array; one ACT exp per mc chunk
  covers both heads [128, 2, 512]; PV accumulates [v|1]^T P into po[65, 512]
  whose 65th row is the softmax denominator; DVE normalizes. The attention
  phase is ACT(exp)-bound (~151us standalone); po is double-buffered so the
  normalize tail never stalls the next unit's PV chain.
- Proj: ot x wp row-shard, partial outputs DMA'd out (host reduces).

All matmul operands are bf16 (PSUM accumulation stays fp32); tolerance is
2e-2 and measured error is ~2.5e-3. (fp8 x/w for QKV was tried and is
numerically dead: pre-matmul quantization noise accumulates through the
1024-deep contraction into ~7% error in v.)

The timed loop build (loop_iters=P) emits a software-pipelined body: the
persistent q/k/v buffers are doubled and each For_i iteration runs
[attn(buf0); proj; qkv->buf1; attn(buf1); proj; qkv->buf0] (2 logical
forwards per For_i iteration, prologue qkv->buf0 before the loop) with
staggered semaphore resets, so the QKV of the next forward overlaps the
proj/output-DMA tail of the current one.
"""

import sys

sys.path.insert(0, "/opt/trn_rl_repo")

import numpy as np


from contextlib import ExitStack

import concourse.bacc as bacc
import concourse.tile as tile
from concourse import mybir

F32 = mybir.dt.float32
BF16 = mybir.dt.bfloat16

N = 2048
C = 1024
HL = 4  # heads per core
D = 64
KC = C // 128  # 8 contraction chunks
NS = N // 512  # 4 n-supers / slabs
MC = N // 128  # 16 m-chunks


def build_attention_nc(
    mm_dtype=BF16,
    loop_iters=None,
    phases=(1, 1, 1),
    stag=True,
    pipe=True,  # software-pipelined double-buffered loop body (loop builds only)
):
    nc = bacc.Bacc(None, target_bir_lowering=False, debug=False)

    MMDT = mm_dtype
    nbuf = 2 if (pipe and loop_iters is not None) else 1

    xt = nc.dram_tensor("xt", [C, N], MMDT, kind="ExternalInput")
    wq = nc.dram_tensor("wq", [C, 2, 128], MMDT, kind="ExternalInput")
    wk = nc.dram_tensor("wk", [C, 2, 128], MMDT, kind="ExternalInput")
    wv = nc.dram_tensor("wv", [C, 256], MMDT, kind="ExternalInput")
    bq = nc.dram_tensor("bq", [2, 128], F32, kind="ExternalInput")
    bk = nc.dram_tensor("bk", [2, 128], F32, kind="ExternalInput")
    bv = nc.dram_tensor("bv", [256], F32, kind="ExternalInput")
    wp = nc.dram_tensor("wp", [256, C], MMDT, kind="ExternalInput")
    onesv = nc.dram_tensor("onesv", [128, 64], MMDT, kind="ExternalInput")
    out = nc.dram_tensor("out", [N, C], F32, kind="ExternalOutput")

    with (
        tile.TileContext(nc) as tc,
        ExitStack() as ctx,
        nc.allow_low_precision(reason="bf16 matmuls within 2e-2 tolerance"),
    ):
        const = ctx.enter_context(tc.tile_pool(name="const", bufs=1))
        persist = ctx.enter_context(tc.tile_pool(name="persist", bufs=1))

        # --- constants / weights ---
        wq_sb = const.tile([128, KC, 256], MMDT)
        wk_sb = const.tile([128, KC, 256], MMDT)
        wv_sb = const.tile([128, KC, 256], MMDT)
        nc.sync.dma_start(out=wq_sb, in_=wq.rearrange("(kc p) j m -> p kc (j m)", p=128))
        nc.sync.dma_start(out=wk_sb, in_=wk.rearrange("(kc p) j m -> p kc (j m)", p=128))
        nc.sync.dma_start(out=wv_sb, in_=wv.rearrange("(kc p) m -> p kc m", p=128))
        bq_sb = const.tile([128, 2], F32)
        bk_sb = const.tile([128, 2], F32)
        nc.sync.dma_start(out=bq_sb, in_=bq.rearrange("j p -> p j"))
        nc.sync.dma_start(out=bk_sb, in_=bk.rearrange("j p -> p j"))
        bv_rep = const.tile([128, 256], F32)
        nc.sync.dma_start(out=bv_rep, in_=bv[:].unsqueeze(0).partition_broadcast(128))
        wp_sb = const.tile([128, 2, C], MMDT)
        nc.sync.dma_start(out=wp_sb, in_=wp.rearrange("(kc p) n -> p kc n", p=128))

        # --- persistent intermediates (nbuf copies for the pipelined loop) ---
        qt_sb = [persist.tile([128, 2, N], MMDT, name=f"qt{b}") for b in range(nbuf)]
        kt_sb = [persist.tile([128, 2, N], MMDT, name=f"kt{b}") for b in range(nbuf)]
        v_sb = [
            persist.tile([128, MC, HL, 65], MMDT, name=f"v{b}") for b in range(nbuf)
        ]  # [m, mc, head, d|1]
        ot_sb = persist.tile([128, 2, N], MMDT)  # [d(2 heads), hd-chunk, n]
        for b in range(nbuf):
            nc.sync.dma_start(
                out=v_sb[b][:, :, :, 64:65],
                in_=onesv.rearrange("p (a b c) -> p a b c", a=MC, b=HL, c=1),
            )
        # Phase-isolation profiling support: when an upstream phase is
        # disabled, zero-init the tiles it would have produced so downstream
        # phases are runnable. No-op for the normal (1,1,1) build.
        if not phases[0]:
            for b in range(nbuf):
                for t in (qt_sb[b], kt_sb[b]):
                    nc.vector.memset(t.bitcast(mybir.dt.uint16), 0)
                nc.vector.memset(v_sb[b][:, :, :, 0:64].bitcast(mybir.dt.uint16), 0)
        if not phases[1] and phases[2]:
            nc.vector.memset(ot_sb.bitcast(mybir.dt.uint16), 0)

        def qkv_gen(b, xt_pool, qkv_ps):
            """Yield closures, each emitting one instruction-group of QKV."""
            for i in range(NS):
                xts = xt_pool.tile([128, KC, 512], MMDT, tag="xts")

                def dma(i=i, xts=xts):
                    nc.sync.dma_start(
                        out=xts,
                        in_=xt.rearrange("(kc p) n -> p kc n", p=128)[
                            :, :, i * 512 : (i + 1) * 512
                        ],
                    )

                yield dma
                for wsb, bsb, dst in (
                    (wq_sb, bq_sb, qt_sb[b]),
                    (wk_sb, bk_sb, kt_sb[b]),
                ):
                    for j in range(2):
                        ps = qkv_ps.tile([128, 512], F32, tag="ps", name=f"qk_ps{i}{j}")

                        def chain(ps=ps, wsb=wsb, bsb=bsb, dst=dst, j=j, i=i, xts=xts):
                            for kc in range(KC):
                                nc.tensor.matmul(
                                    ps,
                                    lhsT=wsb[:, kc, j * 128 : (j + 1) * 128],
                                    rhs=xts[:, kc, :],
                                    start=kc == 0,
                                    stop=kc == KC - 1,
                                )
                            nc.vector.tensor_scalar_add(
                                out=dst[:, j, i * 512 : (i + 1) * 512],
                                in0=ps,
                                scalar1=bsb[:, j : j + 1],
                            )

                        yield chain
                for jj in range(4):
                    ps = qkv_ps.tile([128, 256], F32, tag="psv", name=f"v_ps{i}{jj}")

                    def chainv(ps=ps, jj=jj, i=i, xts=xts):
                        for kc in range(KC):
                            nc.tensor.matmul(
                                ps,
                                lhsT=xts[:, kc, jj * 128 : (jj + 1) * 128],
                                rhs=wv_sb[:, kc, :],
                                start=kc == 0,
                                stop=kc == KC - 1,
                            )
                        mc = i * 4 + jj
                        nc.vector.tensor_add(
                            out=v_sb[b][:, mc, :, 0:64],
                            in0=ps.rearrange("p (h d) -> p h d", h=HL),
                            in1=bv_rep.rearrange("p (h d) -> p h d", h=HL),
                        )

                    yield chainv

        def phase_qkv(b):
            with (
                tc.tile_pool(name="xt_pool", bufs=2) as xt_pool,
                tc.tile_pool(name="qkv_ps", bufs=4, space="PSUM") as qkv_ps,
            ):
                for item in qkv_gen(b, xt_pool, qkv_ps):
                    item()

        def phase_attn(b):
            # Head-pair processing: heads (2j, 2j+1) live in partition halves
            # 0:64 / 64:128 of qt/kt, so their K=64 St matmuls occupy disjoint
            # PE row groups (tile_position (0,0) / (64,0)) and stream
            # concurrently. Ring slot = one mc chunk x both heads
            # [128, 2, 512]; exp covers both heads in one ACT instruction.
            with (
                tc.tile_pool(name="st_ps", bufs=1, space="PSUM") as st_ps,
                tc.tile_pool(name="o_ps", bufs=2, space="PSUM") as o_ps,
                tc.tile_pool(name="p_pool", bufs=4) as p_pool,
                tc.tile_pool(name="r_pool", bufs=4) as r_pool,
            ):
                rings = [
                    st_ps.tile([128, 2, 512], F32, tag=f"ring{u}", name=f"ring{u}")
                    for u in range(2)
                ]

                # Flat global slot pipeline across all (s, j, mc): PV lags one
                # slot globally, so a new unit's St+exp issue before the
                # previous unit's last PV pair — no ACT bubble at unit
                # boundaries. The unit tail (normalize) is emitted right
                # after that unit's last PV retires.
                slots = [
                    (s, j, mc) for s in range(NS) for j in range(2) for mc in range(MC)
                ]
                po_of = {}

                def emit_pv(item):
                    s, j, mc, pt = item
                    poA, poB = po_of[(s, j)]
                    for hh, po in ((0, poA), (1, poB)):
                        nc.tensor.matmul(
                            po[0:65, :],
                            lhsT=v_sb[b][:, mc, 2 * j + hh, :],
                            rhs=pt[:, hh, :],
                            start=mc == 0,
                            stop=mc == MC - 1,
                        )

                def emit_tail(s, j):
                    poA, poB = po_of.pop((s, j))
                    for hh, po in ((0, poA), (1, poB)):
                        recip = r_pool.tile([1, 512], F32, tag=f"recip{hh}")
                        nc.vector.reciprocal(out=recip, in_=po[64:65, :])
                        rden_sb = r_pool.tile(
                            [64, 512], F32, tag=f"rden{hh}", name=f"rd_s{s}j{j}h{hh}"
                        )
                        nc.gpsimd.partition_broadcast(rden_sb, recip)
                        nc.vector.tensor_mul(
                            out=ot_sb[
                                hh * 64 : hh * 64 + 64,
                                j,
                                s * 512 : (s + 1) * 512,
                            ],
                            in0=po[0:64, :],
                            in1=rden_sb,
                        )

                prev = None
                for gi, (s, j, mc) in enumerate(slots):
                    if mc == 0:
                        po_of[(s, j)] = (
                            o_ps.tile([128, 512], F32, tag="poA", name=f"poA_s{s}j{j}"),
                            o_ps.tile([128, 512], F32, tag="poB", name=f"poB_s{s}j{j}"),
                        )
                    ring = rings[gi % 2]
                    for hh in range(2):
                        base = hh * 64
                        nc.tensor.matmul(
                            ring[:, hh, :],
                            lhsT=kt_sb[b][
                                base : base + 64, j, mc * 128 : (mc + 1) * 128
                            ],
                            rhs=qt_sb[b][
                                base : base + 64, j, s * 512 : (s + 1) * 512
                            ],
                            start=True,
                            stop=True,
                        )
                    pt = p_pool.tile([128, 2, 512], MMDT, tag="pt")
                    nc.scalar.activation(
                        out=pt,
                        in_=ring,
                        func=mybir.ActivationFunctionType.Exp,
                    )
                    if prev is not None:
                        emit_pv(prev)
                        ps_, pj_, pmc_ = prev[0], prev[1], prev[2]
                        if pmc_ == MC - 1:
                            emit_tail(ps_, pj_)
                    prev = (s, j, mc, pt)
                emit_pv(prev)
                emit_tail(prev[0], prev[1])

        def proj_gen(pj_ps, out_pool):
            """Yield closures, each emitting one [128, 512] proj chunk.
            (Matmul PSUM output is limited to one bank = 512 fp32 free.)"""
            for nt in range(MC):
                for cc in range(2):
                    ps = pj_ps.tile([128, 512], F32, tag="pjps", name=f"pj{nt}{cc}")

                    def unit(ps=ps, nt=nt, cc=cc):
                        for hdc in range(2):
                            nc.tensor.matmul(
                                ps,
                                lhsT=ot_sb[:, hdc, nt * 128 : (nt + 1) * 128],
                                rhs=wp_sb[:, hdc, cc * 512 : (cc + 1) * 512],
                                start=hdc == 0,
                                stop=hdc == 1,
                            )
                        so = out_pool.tile([128, 512], F32, tag="so")
                        nc.vector.tensor_copy(out=so, in_=ps)
                        nc.sync.dma_start(
                            out=out[
                                nt * 128 : (nt + 1) * 128, cc * 512 : (cc + 1) * 512
                            ],
                            in_=so,
                        )

                    yield unit

        def phase_proj():
            with (
                tc.tile_pool(name="pj_ps", bufs=4, space="PSUM") as pj_ps,
                tc.tile_pool(name="out_pool", bufs=3) as out_pool,
            ):
                for item in proj_gen(pj_ps, out_pool):
                    item()

        def phase_tail(cur, nxt):
            # proj(cur) and qkv(nxt) are independent (proj reads ot/wp, qkv
            # writes the other persistent buffer): interleave their emission
            # so PE matmuls, DVE drains/copies, and DMA pipeline instead of
            # running the two phases back-to-back.
            with (
                tc.tile_pool(name="xt_pool", bufs=2) as xt_pool,
                tc.tile_pool(name="qkv_ps", bufs=2, space="PSUM") as qkv_ps,
                tc.tile_pool(name="pj_ps", bufs=4, space="PSUM") as pj_ps,
                tc.tile_pool(name="out_pool", bufs=3) as out_pool,
            ):
                pj = list(proj_gen(pj_ps, out_pool))
                qk = list(qkv_gen(nxt, xt_pool, qkv_ps))
                na, nb = len(pj), len(qk)
                i = j = 0
                while i < na or j < nb:
                    if j < nb and (i >= na or j * na <= i * nb):
                        qk[j]()
                        j += 1
                    else:
                        pj[i]()
                        i += 1

        def body(b):
            if phases[0]:
                phase_qkv(b)
            if phases[1]:
                phase_attn(b)
            if phases[2]:
                phase_proj()

        if loop_iters is None:
            body(0)
        elif not pipe:
            with tc.For_i(0, loop_iters, 1, staggered_reset=stag):
                body(0)
        else:
            # Software-pipelined: prologue fills buf0; each For_i iteration
            # consumes one buffer and refills the other, twice.
            if phases[0]:
                phase_qkv(0)
            with tc.For_i(0, loop_iters, 1, staggered_reset=stag):
                for half in (0, 1):
                    if phases == (1, 1, 1):
                        phase_attn(half)
                        phase_tail(half, 1 - half)
                    else:
                        if phases[1]:
                            phase_attn(half)
                        if phases[2]:
                            phase_proj()
                        if phases[0]:
                            phase_qkv(1 - half)

    nc.compile()
    return nc


EMBED_DIM = 1024
NUM_HEADS = 16
HEAD_DIM = 64
HPC = 4

_CACHE = {}


def _make_in_maps(x, w_qkv, b_qkv, w_proj):
    import ml_dtypes

    MM_NP = ml_dtypes.bfloat16
    scale = HEAD_DIM ** -0.5
    xts = [np.ascontiguousarray(x[b].T).astype(MM_NP) for b in range(2)]
    ones = np.ones((128, 64), MM_NP)
    in_maps = []
    for core in range(8):
        b, g = core // 4, core % 4
        cols = slice(g * HPC * HEAD_DIM, (g + 1) * HPC * HEAD_DIM)
        wq = (w_qkv[:, 0:C][:, cols] * scale).astype(MM_NP)
        wk = w_qkv[:, C : 2 * C][:, cols].astype(MM_NP)
        wv = w_qkv[:, 2 * C : 3 * C][:, cols].astype(MM_NP)
        bq = (b_qkv[0:C][cols] * scale).astype(np.float32)
        bk = b_qkv[C : 2 * C][cols].astype(np.float32)
        bvv = b_qkv[2 * C : 3 * C][cols].astype(np.float32)
        wp = np.ascontiguousarray(w_proj[cols.start : cols.stop, :]).astype(MM_NP)
        in_maps.append(
            {
                "xt": xts[b],
                "wq": np.ascontiguousarray(wq.reshape(C, 2, 128)),
                "wk": np.ascontiguousarray(wk.reshape(C, 2, 128)),
                "wv": np.ascontiguousarray(wv),
                "bq": np.ascontiguousarray(bq.reshape(2, 128)),
                "bk": np.ascontiguousarray(bk.reshape(2, 128)),
                "bv": np.ascontiguousarray(bvv),
                "wp": wp,
                "onesv": ones,
            }
        )
    return in_maps


def kernel(x, w_qkv, b_qkv, w_proj, b_proj):
    from concourse.bass_utils import run_bass_kernel_spmd

    x = np.asarray(x)
    w_qkv = np.asarray(w_qkv)
    b_qkv = np.asarray(b_qkv)
    w_proj = np.asarray(w_proj)
    b_proj = np.asarray(b_proj)

    if "nc" not in _CACHE:
        _CACHE["nc"] = build_attention_nc()
    nc = _CACHE["nc"]

    in_maps = _make_in_maps(x, w_qkv, b_qkv, w_proj)
    res = run_bass_kernel_spmd(nc, in_maps, core_ids=list(range(8)))

    outs = []
    for b in range(2):
        acc = res.results[b * 4]["out"].astype(np.float32).copy()
        for g in range(1, 4):
            acc += res.results[b * 4 + g]["out"]
        outs.append(acc)
    return (np.stack(outs) + b_proj.astype(np.float32)).astype(np.float32)


# revision 31
# speedup vs baseline: 1.1216x; 1.0022x over previous
"""Multi-head attention block (B=2, N=2048, C=1024, H=16, D=64) on 8
Trainium2 NeuronCores.

Sharding: core c -> batch b = c//4, head-group g = c%4 (tensor-parallel over
heads within a batch, 4 heads per core). QKV weights are column-sharded,
w_proj row-sharded; each core emits a partial [N, C] projection which the host
sums per batch (row-parallel reduce done on host) and then adds b_proj.

Per-core kernel structure (build_attention_nc):
- QKV: xt slabs (512 tokens) x column-sharded weights -> q,k (transposed
  layout, head pairs split across partition halves) and v (token-major, with
  a ones-column appended per head for the softmax denominator).
- Attention: for each (slab, head-pair), St = K^T Q for both heads issue as
  K=64 matmuls in opposite PE row groups (tile_position (0,0) / (64,0)) so
  they stream concurrently through the array; one ACT exp per mc chunk
  covers both heads [128, 2, 512]; PV accumulates [v|1]^T P into po[65, 512]
  whose 65th row is the softmax denominator; DVE normalizes. The attention
  phase is ACT(exp)-bound (~151us standalone); po is double-buffered so the
  normalize tail never stalls the next unit's PV chain.
- Proj: ot x wp row-shard, partial outputs DMA'd out (host reduces).

All matmul operands are bf16 (PSUM accumulation stays fp32); tolerance is
2e-2 and measured error is ~2.5e-3. (fp8 x/w for QKV was tried and is
numerically dead: pre-matmul quantization noise accumulates through the
1024-deep contraction into ~7% error in v.)

The timed loop build (loop_iters=P) emits a software-pipelined body: the
persistent q/k/v buffers are doubled and each For_i iteration runs
[attn(buf0); proj; qkv->buf1; attn(buf1); proj; qkv->buf0] (2 logical
forwards per For_i iteration, prologue qkv->buf0 before the loop) with
staggered semaphore resets, so the QKV of the next forward overlaps the
proj/output-DMA tail of the current one.
"""

import sys

sys.path.insert(0, "/opt/trn_rl_repo")

import numpy as np


from contextlib import ExitStack

import concourse.bacc as bacc
import concourse.tile as tile
from concourse import mybir

F32 = mybir.dt.float32
BF16 = mybir.dt.bfloat16

N = 2048
C = 1024
HL = 4  # heads per core
D = 64
KC = C // 128  # 8 contraction chunks
NS = N // 512  # 4 n-supers / slabs
MC = N // 128  # 16 m-chunks


def build_attention_nc(
    mm_dtype=BF16,
    loop_iters=None,
    phases=(1, 1, 1),
    stag=True,
    pipe=True,  # software-pipelined double-buffered loop body (loop builds only)
):
    nc = bacc.Bacc(None, target_bir_lowering=False, debug=False)

    MMDT = mm_dtype
    nbuf = 2 if (pipe and loop_iters is not None) else 1

    xt = nc.dram_tensor("xt", [C, N], MMDT, kind="ExternalInput")
    wq = nc.dram_tensor("wq", [C, 2, 128], MMDT, kind="ExternalInput")
    wk = nc.dram_tensor("wk", [C, 2, 128], MMDT, kind="ExternalInput")
    wv = nc.dram_tensor("wv", [C, 256], MMDT, kind="ExternalInput")
    bq = nc.dram_tensor("bq", [2, 128], F32, kind="ExternalInput")
    bk = nc.dram_tensor("bk", [2, 128], F32, kind="ExternalInput")
    bv = nc.dram_tensor("bv", [256], F32, kind="ExternalInput")
    wp = nc.dram_tensor("wp", [256, C], MMDT, kind="ExternalInput")
    onesv = nc.dram_tensor("onesv", [128, 64], MMDT, kind="ExternalInput")
    out = nc.dram_tensor("out", [N, C], F32, kind="ExternalOutput")

    with (
        tile.TileContext(nc) as tc,
        ExitStack() as ctx,
        nc.allow_low_precision(reason="bf16 matmuls within 2e-2 tolerance"),
    ):
        const = ctx.enter_context(tc.tile_pool(name="const", bufs=1))
        persist = ctx.enter_context(tc.tile_pool(name="persist", bufs=1))

        # --- constants / weights ---
        wq_sb = const.tile([128, KC, 256], MMDT)
        wk_sb = const.tile([128, KC, 256], MMDT)
        wv_sb = const.tile([128, KC, 256], MMDT)
        nc.sync.dma_start(out=wq_sb, in_=wq.rearrange("(kc p) j m -> p kc (j m)", p=128))
        nc.sync.dma_start(out=wk_sb, in_=wk.rearrange("(kc p) j m -> p kc (j m)", p=128))
        nc.sync.dma_start(out=wv_sb, in_=wv.rearrange("(kc p) m -> p kc m", p=128))
        bq_sb = const.tile([128, 2], F32)
        bk_sb = const.tile([128, 2], F32)
        nc.sync.dma_start(out=bq_sb, in_=bq.rearrange("j p -> p j"))
        nc.sync.dma_start(out=bk_sb, in_=bk.rearrange("j p -> p j"))
        bv_rep = const.tile([128, 256], F32)
        nc.sync.dma_start(out=bv_rep, in_=bv[:].unsqueeze(0).partition_broadcast(128))
        wp_sb = const.tile([128, 2, C], MMDT)
        nc.sync.dma_start(out=wp_sb, in_=wp.rearrange("(kc p) n -> p kc n", p=128))

        # --- persistent intermediates (nbuf copies for the pipelined loop) ---
        qt_sb = [persist.tile([128, 2, N], MMDT, name=f"qt{b}") for b in range(nbuf)]
        kt_sb = [persist.tile([128, 2, N], MMDT, name=f"kt{b}") for b in range(nbuf)]
        v_sb = [
            persist.tile([128, MC, HL, 65], MMDT, name=f"v{b}") for b in range(nbuf)
        ]  # [m, mc, head, d|1]
        ot_sb = persist.tile([128, 2, N], MMDT)  # [d(2 heads), hd-chunk, n]
        for b in range(nbuf):
            nc.sync.dma_start(
                out=v_sb[b][:, :, :, 64:65],
                in_=onesv.rearrange("p (a b c) -> p a b c", a=MC, b=HL, c=1),
            )
        # Phase-isolation profiling support: when an upstream phase is
        # disabled, zero-init the tiles it would have produced so downstream
        # phases are runnable. No-op for the normal (1,1,1) build.
        if not phases[0]:
            for b in range(nbuf):
                for t in (qt_sb[b], kt_sb[b]):
                    nc.vector.memset(t.bitcast(mybir.dt.uint16), 0)
                nc.vector.memset(v_sb[b][:, :, :, 0:64].bitcast(mybir.dt.uint16), 0)
        if not phases[1] and phases[2]:
            nc.vector.memset(ot_sb.bitcast(mybir.dt.uint16), 0)

        def qkv_gen(b, xt_pool, qkv_ps):
            """Yield (is_dma, closure) pairs, each closure emitting one
            instruction-group of QKV."""
            for i in range(NS):
                xts = xt_pool.tile([128, KC, 512], MMDT, tag="xts")

                def dma(i=i, xts=xts):
                    nc.sync.dma_start(
                        out=xts,
                        in_=xt.rearrange("(kc p) n -> p kc n", p=128)[
                            :, :, i * 512 : (i + 1) * 512
                        ],
                    )

                yield True, dma
                for wsb, bsb, dst in (
                    (wq_sb, bq_sb, qt_sb[b]),
                    (wk_sb, bk_sb, kt_sb[b]),
                ):
                    for j in range(2):
                        ps = qkv_ps.tile([128, 512], F32, tag="ps", name=f"qk_ps{i}{j}")

                        def chain(ps=ps, wsb=wsb, bsb=bsb, dst=dst, j=j, i=i, xts=xts):
                            for kc in range(KC):
                                nc.tensor.matmul(
                                    ps,
                                    lhsT=wsb[:, kc, j * 128 : (j + 1) * 128],
                                    rhs=xts[:, kc, :],
                                    start=kc == 0,
                                    stop=kc == KC - 1,
                                )
                            nc.vector.tensor_scalar_add(
                                out=dst[:, j, i * 512 : (i + 1) * 512],
                                in0=ps,
                                scalar1=bsb[:, j : j + 1],
                            )

                        yield False, chain
                for jj in range(4):
                    ps = qkv_ps.tile([128, 256], F32, tag="psv", name=f"v_ps{i}{jj}")

                    def chainv(ps=ps, jj=jj, i=i, xts=xts):
                        for kc in range(KC):
                            nc.tensor.matmul(
                                ps,
                                lhsT=xts[:, kc, jj * 128 : (jj + 1) * 128],
                                rhs=wv_sb[:, kc, :],
                                start=kc == 0,
                                stop=kc == KC - 1,
                            )
                        mc = i * 4 + jj
                        nc.vector.tensor_add(
                            out=v_sb[b][:, mc, :, 0:64],
                            in0=ps.rearrange("p (h d) -> p h d", h=HL),
                            in1=bv_rep.rearrange("p (h d) -> p h d", h=HL),
                        )

                    yield False, chainv

        def phase_qkv(b):
            with (
                tc.tile_pool(name="xt_pool", bufs=2) as xt_pool,
                tc.tile_pool(name="qkv_ps", bufs=4, space="PSUM") as qkv_ps,
            ):
                for _, item in qkv_gen(b, xt_pool, qkv_ps):
                    item()

        def phase_attn(b):
            # Head-pair processing: heads (2j, 2j+1) live in partition halves
            # 0:64 / 64:128 of qt/kt, so their K=64 St matmuls occupy disjoint
            # PE row groups (tile_position (0,0) / (64,0)) and stream
            # concurrently. Ring slot = one mc chunk x both heads
            # [128, 2, 512]; exp covers both heads in one ACT instruction.
            with (
                tc.tile_pool(name="st_ps", bufs=1, space="PSUM") as st_ps,
                tc.tile_pool(name="o_ps", bufs=2, space="PSUM") as o_ps,
                tc.tile_pool(name="p_pool", bufs=4) as p_pool,
                tc.tile_pool(name="r_pool", bufs=4) as r_pool,
            ):
                rings = [
                    st_ps.tile([128, 2, 512], F32, tag=f"ring{u}", name=f"ring{u}")
                    for u in range(2)
                ]

                # Flat global slot pipeline across all (s, j, mc): PV lags one
                # slot globally, so a new unit's St+exp issue before the
                # previous unit's last PV pair — no ACT bubble at unit
                # boundaries. The unit tail (normalize) is emitted right
                # after that unit's last PV retires.
                slots = [
                    (s, j, mc) for s in range(NS) for j in range(2) for mc in range(MC)
                ]
                po_of = {}

                def emit_pv(item):
                    s, j, mc, pt = item
                    poA, poB = po_of[(s, j)]
                    for hh, po in ((0, poA), (1, poB)):
                        nc.tensor.matmul(
                            po[0:65, :],
                            lhsT=v_sb[b][:, mc, 2 * j + hh, :],
                            rhs=pt[:, hh, :],
                            start=mc == 0,
                            stop=mc == MC - 1,
                        )

                def emit_tail(s, j):
                    poA, poB = po_of.pop((s, j))
                    for hh, po in ((0, poA), (1, poB)):
                        recip = r_pool.tile([1, 512], F32, tag=f"recip{hh}")
                        nc.vector.reciprocal(out=recip, in_=po[64:65, :])
                        rden_sb = r_pool.tile(
                            [64, 512], F32, tag=f"rden{hh}", name=f"rd_s{s}j{j}h{hh}"
                        )
                        nc.gpsimd.partition_broadcast(rden_sb, recip)
                        nc.vector.tensor_mul(
                            out=ot_sb[
                                hh * 64 : hh * 64 + 64,
                                j,
                                s * 512 : (s + 1) * 512,
                            ],
                            in0=po[0:64, :],
                            in1=rden_sb,
                        )

                prev = None
                for gi, (s, j, mc) in enumerate(slots):
                    if mc == 0:
                        po_of[(s, j)] = (
                            o_ps.tile([128, 512], F32, tag="poA", name=f"poA_s{s}j{j}"),
                            o_ps.tile([128, 512], F32, tag="poB", name=f"poB_s{s}j{j}"),
                        )
                    ring = rings[gi % 2]
                    for hh in range(2):
                        base = hh * 64
                        nc.tensor.matmul(
                            ring[:, hh, :],
                            lhsT=kt_sb[b][
                                base : base + 64, j, mc * 128 : (mc + 1) * 128
                            ],
                            rhs=qt_sb[b][
                                base : base + 64, j, s * 512 : (s + 1) * 512
                            ],
                            start=True,
                            stop=True,
                        )
                    pt = p_pool.tile([128, 2, 512], MMDT, tag="pt")
                    nc.scalar.activation(
                        out=pt,
                        in_=ring,
                        func=mybir.ActivationFunctionType.Exp,
                    )
                    if prev is not None:
                        emit_pv(prev)
                        ps_, pj_, pmc_ = prev[0], prev[1], prev[2]
                        if pmc_ == MC - 1:
                            emit_tail(ps_, pj_)
                    prev = (s, j, mc, pt)
                emit_pv(prev)
                emit_tail(prev[0], prev[1])

        def proj_gen(pj_ps, out_pool):
            """Yield closures, each emitting one [128, 512] proj chunk.
            (Matmul PSUM output is limited to one bank = 512 fp32 free.)"""
            for nt in range(MC):
                for cc in range(2):
                    ps = pj_ps.tile([128, 512], F32, tag="pjps", name=f"pj{nt}{cc}")

                    def unit(ps=ps, nt=nt, cc=cc):
                        for hdc in range(2):
                            nc.tensor.matmul(
                                ps,
                                lhsT=ot_sb[:, hdc, nt * 128 : (nt + 1) * 128],
                                rhs=wp_sb[:, hdc, cc * 512 : (cc + 1) * 512],
                                start=hdc == 0,
                                stop=hdc == 1,
                            )
                        so = out_pool.tile([128, 512], F32, tag="so")
                        nc.vector.tensor_copy(out=so, in_=ps)
                        nc.sync.dma_start(
                            out=out[
                                nt * 128 : (nt + 1) * 128, cc * 512 : (cc + 1) * 512
                            ],
                            in_=so,
                        )

                    yield unit

        def phase_proj():
            with (
                tc.tile_pool(name="pj_ps", bufs=4, space="PSUM") as pj_ps,
                tc.tile_pool(name="out_pool", bufs=3) as out_pool,
            ):
                for item in proj_gen(pj_ps, out_pool):
                    item()

        def phase_tail(cur, nxt):
            # proj(cur) and qkv(nxt) are independent (proj reads ot/wp, qkv
            # writes the other persistent buffer): interleave their emission
            # so PE matmuls, DVE drains/copies, and DMA pipeline instead of
            # running the two phases back-to-back.
            with (
                tc.tile_pool(name="xt_pool", bufs=2) as xt_pool,
                tc.tile_pool(name="qkv_ps", bufs=2, space="PSUM") as qkv_ps,
                tc.tile_pool(name="pj_ps", bufs=4, space="PSUM") as pj_ps,
                tc.tile_pool(name="out_pool", bufs=3) as out_pool,
            ):
                pj = list(proj_gen(pj_ps, out_pool))
                qk = [fn for _, fn in qkv_gen(nxt, xt_pool, qkv_ps)]
                na, nb = len(pj), len(qk)
                i = j = 0
                while i < na or j < nb:
                    if j < nb and (i >= na or j * na <= i * nb):
                        qk[j]()
                        j += 1
                    else:
                        pj[i]()
                        i += 1

        def body(b):
            if phases[0]:
                phase_qkv(b)
            if phases[1]:
                phase_attn(b)
            if phases[2]:
                phase_proj()

        if loop_iters is None:
            body(0)
        elif not pipe:
            with tc.For_i(0, loop_iters, 1, staggered_reset=stag):
                body(0)
        else:
            # Software-pipelined: prologue fills buf0; each For_i iteration
            # consumes one buffer and refills the other, twice.
            if phases[0]:
                phase_qkv(0)
            with tc.For_i(0, loop_iters, 1, staggered_reset=stag):
                for half in (0, 1):
                    if phases == (1, 1, 1):
                        phase_attn(half)
                        phase_tail(half, 1 - half)
                    else:
                        if phases[1]:
                            phase_attn(half)
                        if phases[2]:
                            phase_proj()
                        if phases[0]:
                            phase_qkv(1 - half)

    nc.compile()
    return nc


EMBED_DIM = 1024
NUM_HEADS = 16
HEAD_DIM = 64
HPC = 4

_CACHE = {}


def _make_in_maps(x, w_qkv, b_qkv, w_proj):
    import ml_dtypes

    MM_NP = ml_dtypes.bfloat16
    scale = HEAD_DIM ** -0.5
    xts = [np.ascontiguousarray(x[b].T).astype(MM_NP) for b in range(2)]
    ones = np.ones((128, 64), MM_NP)
    in_maps = []
    for core in range(8):
        b, g = core // 4, core % 4
        cols = slice(g * HPC * HEAD_DIM, (g + 1) * HPC * HEAD_DIM)
        wq = (w_qkv[:, 0:C][:, cols] * scale).astype(MM_NP)
        wk = w_qkv[:, C : 2 * C][:, cols].astype(MM_NP)
        wv = w_qkv[:, 2 * C : 3 * C][:, cols].astype(MM_NP)
        bq = (b_qkv[0:C][cols] * scale).astype(np.float32)
        bk = b_qkv[C : 2 * C][cols].astype(np.float32)
        bvv = b_qkv[2 * C : 3 * C][cols].astype(np.float32)
        wp = np.ascontiguousarray(w_proj[cols.start : cols.stop, :]).astype(MM_NP)
        in_maps.append(
            {
                "xt": xts[b],
                "wq": np.ascontiguousarray(wq.reshape(C, 2, 128)),
                "wk": np.ascontiguousarray(wk.reshape(C, 2, 128)),
                "wv": np.ascontiguousarray(wv),
                "bq": np.ascontiguousarray(bq.reshape(2, 128)),
                "bk": np.ascontiguousarray(bk.reshape(2, 128)),
                "bv": np.ascontiguousarray(bvv),
                "wp": wp,
                "onesv": ones,
            }
        )
    return in_maps


def kernel(x, w_qkv, b_qkv, w_proj, b_proj):
    from concourse.bass_utils import run_bass_kernel_spmd

    x = np.asarray(x)
    w_qkv = np.asarray(w_qkv)
    b_qkv = np.asarray(b_qkv)
    w_proj = np.asarray(w_proj)
    b_proj = np.asarray(b_proj)

    if "nc" not in _CACHE:
        _CACHE["nc"] = build_attention_nc()
    nc = _CACHE["nc"]

    in_maps = _make_in_maps(x, w_qkv, b_qkv, w_proj)
    res = run_bass_kernel_spmd(nc, in_maps, core_ids=list(range(8)))

    outs = []
    for b in range(2):
        acc = res.results[b * 4]["out"].astype(np.float32).copy()
        for g in range(1, 4):
            acc += res.results[b * 4 + g]["out"]
        outs.append(acc)
    return (np.stack(outs) + b_proj.astype(np.float32)).astype(np.float32)
